# revision 36
# baseline (speedup 1.0000x reference)
"""Trainium2 Bass kernel for the moe_routing ensemble-MLP problem.

Math (reference):
    inp_i = concat(states_i, actions_i)                 # (E, B, 40)
    h1    = leaky(inp_i @ W0_j)                         # per (i, j)
    h2    = leaky(h1 @ W1_j)
    p_ij  = h2 @ W2_j                                   # (E, E, B, 32)
    mean_out[j] = mean_i(p_ij) + states[j]
    var_out[j]  = var_i(p_ij, ddof=1)

Strategy: data-parallel over B across 8 cores (B_loc = 2048/core).
Per core, everything is computed feature-major ([feat, tokens] tiles).

v3 (the graded configuration) — all-bf16 matmuls:
  - On real TRN2 hardware fp32/f32r matmuls run at 4 cycles/row; bf16
    runs at 1. The v2 baseline was PE-bound at ~684us/pass purely from
    f32r matmul rate. v3 runs mm0/mm1/mm2 with bf16 stationaries and
    bf16 moving operands (fp32 PSUM accumulation), measured end-to-end
    rel-err ~5e-3 vs the 2e-2 budget.
  - Inputs arrive host-pretransposed as bf16 [DIN, B_loc] per slice, so
    the on-device PE-transpose prologue is gone.
  - leaky on layer 1 is relu-decomposed (leaky(x) = 0.99 relu(x) +
    0.01 x): r1 = relu(g1) is ONE DVE tensor_scalar_max per 512-chunk
    (DVE cannot do single-op leaky from PSUM: dual-PSUM reads are
    illegal), and the linear term is a host-prebuilt fp8 fused weight
    W01 = 0.01*(W0@W1) applied to an fp8 input copy with DoubleRow
    perf mode (half PE cost).  Layer-2 leaky is ONE exact ACT Prelu
    per [128,1024] PSUM chunk.
  - preds: 4 accumulating matmuls per pair stack the 4 token-quarters
    at partition block 32t via a zero-padded stationary (w2z, built on
    host), giving pp = [128, 512] covering all 2048 tokens.
  - pp drain: sum_i preds via DVE copy (i=0) / scalar_tensor_tensor
    add (i>0) STRAIGHT from PSUM; sum_i preds^2 via ACT Square with the
    1/(E-1) scale folded in (i=0 writes the accumulator directly) +
    Pool add.
  - finalize per j needs no separate mean tile: the var mean^2 term is
    ACT Square(s * sum) with s = 1/sqrt(E(E-1)); var is a Pool
    tensor_tensor subtract, mean a DVE STT with the preloaded states
    replica; output flip via DVE 32x32 stream transpose + strided DMA
    store (as v2).
  - emission order per steady-state pair p (PE is the bottleneck, its
    in-order queue must never wait on a drain; drains must be emitted
    AFTER mm1 so they never head-of-line-block DVE/ACT in front of
    relu/leaky2 -- DRAIN_IN_B=True measured ~2x slower on HW):
      A(p): mm0 x4        B(p-1): leaky2 + mm2        C(p): W01 + relu
      D(p): mm1           E(p-1): sum, sq, sumsq, fin slots
  - PSUM: g1 3 bufs x [128,512] + g2 2 bufs x [128,1024] + pp 1 buf
    = 3 + 4 + 1 = 8 banks.
  - slices 2..7 input DMAs are emitted lazily inside the pair loop so
    the first pairs' semaphore work isn't behind the full input stream
    in the finite SP queue (sim-neutral, helped the best HW runs).
  - SBUF pools run deep (hs 12 / h2p 4 / psq 4 / fin 3 / fint 2):
    strictly dependency-relaxing, and HW punishes short queues more
    than the sim does.
  - measured (median repeat-delta, x1.10 sim-calibrated prologue):
    207.5-260.8us across sessions (median ~238us) vs the 800.7us f32r
    baseline; rel err 3.9e-3 (budget 2e-2). Sim floor for this design
    is ~217us/pass: PE 191 / ACT 201 / DVE 200 busy, 8/8 PSUM banks,
    minimal drain op-count given PSUM is DVE/ACT-readable only.
Non-zero-bias inputs fall back to the v1 builder (build_kernel_v1).
"""

import math

import numpy as np

import concourse.bass as bass
import concourse.bacc as bacc
import concourse.mybir as mybir
import concourse.tile as tile
from concourse.bass_utils import run_bass_kernel_spmd

F32 = mybir.dt.float32
F32R = mybir.dt.float32r
BF16 = mybir.dt.bfloat16

E, DS, DA, H = 8, 32, 8, 128
DIN = DS + DA          # 40
B = 16384
N_CORES = 8
B_LOC = B // N_CORES   # 2048
QT = 512               # token quarter = drain granularity
NQ = B_LOC // QT       # 4
CH = 1024              # layer-2 pipeline chunk
NCH = B_LOC // CH      # 2
SLOPE = 0.01

G1N = 512     # g1 PSUM chunk width (DVE relu granularity)
G1B = 3
G2N = 1024    # g2 PSUM chunk width (ACT leaky granularity)
G2B = 2
PPB = 1       # pp pool bufs
RELU_ACT_MOD = 6    # every k-th relu chunk routed to ACT (0 = all DVE)
DRAIN_IN_B = False  # emit pp drains (sum/sq/sumsq) right after mm2
SWAP_BC = False     # emit C (W01+relu) before B (leaky2+mm2)
PP_SHARE_G1 = False  # allocate pp from the g1 pool/tag (saves PSUM banks)
W01_IN_D = False    # emit each W01 right before its mm1 chunk (delays the
                    # g2-bank WAR wait to later in the PE queue)
HS_B, H2P_B, PSQ_B, FIN_B, FINT_B = 12, 4, 4, 3, 2
MEAN_ON_POOL = False  # mean_st via 2 Pool TT ops instead of 1 DVE STT


def build_kernel_v3(repeat: int = 1):
    pp_banks = 0 if PP_SHARE_G1 else PPB
    assert (G1N // QT) * G1B + (G2N // QT) * G2B + pp_banks <= 8, "PSUM banks"
    nc = bacc.Bacc("TRN2")

    inpT_d = nc.dram_tensor("inpT", [E, DIN, B_LOC], BF16,
                            kind="ExternalInput")
    statesT_d = nc.dram_tensor("statesT", [E, DS, B_LOC], F32,
                               kind="ExternalInput")
    w0_d = nc.dram_tensor("w0s", [DIN, E * H], BF16, kind="ExternalInput")
    w1_d = nc.dram_tensor("w1s", [H, E * H], BF16, kind="ExternalInput")
    w2z_d = nc.dram_tensor("w2z", [H, E * 4 * H], BF16, kind="ExternalInput")
    w8_d = nc.dram_tensor("w8", [DIN, E * 2 * H], mybir.dt.float8e4,
                          kind="ExternalInput")
    inp8_d = nc.dram_tensor("inp8", [E, DIN, 2 * B_LOC],
                            mybir.dt.float8e5, kind="ExternalInput")
    mean_d = nc.dram_tensor("mean_out", [E, B_LOC, DS], F32, kind="ExternalOutput")
    var_d = nc.dram_tensor("var_out", [E, B_LOC, DS], F32, kind="ExternalOutput")

    NG1 = B_LOC // G1N  # 4
    NG2 = B_LOC // G2N  # 2
    ctr = {"relu": 0}

    with tile.TileContext(nc) as tc:
        with (
            tc.tile_pool(name="wpool", bufs=1) as wpool,
            tc.tile_pool(name="big", bufs=1) as big,
        ):
            # ---- static weights / inputs (all host-prepacked) ----
            # DMA order matters: the first pairs need w0s/inpT[0]/inp8[0]/
            # w1s/w8s promptly; w2z (1MB) is not needed until the first
            # mm2 (~3us in) and the remaining input slices stream behind.
            w0s = wpool.tile([DIN, E * H], BF16)
            w1s = wpool.tile([H, E * H], BF16)       # holds 0.99*W1
            w2z = wpool.tile([H, E * 4 * H], BF16)   # zero-padded W2 blocks
            w8s = wpool.tile([DIN, E * 2 * H], mybir.dt.float8e4)

            sumsq_acc = big.tile([128, E * QT], F32)
            sum_acc = big.tile([128, E * QT], F32)
            eighth = None
            if MEAN_ON_POOL:
                eighth = big.tile([128, QT], F32, name="eighth")
                nc.gpsimd.memset(eighth[:, :], 1.0 / E)
            inpT = [
                big.tile([DIN, B_LOC], BF16, tag=f"inpT{i}", name=f"inpT{i}")
                for i in range(E)
            ]
            inp8 = [
                big.tile([DIN, 2 * B_LOC], mybir.dt.float8e5,
                         tag=f"inp8_{i}", name=f"inp8_{i}")
                for i in range(E)
            ]
            nc.sync.dma_start(w0s[:, :], w0_d[:, :])
            nc.sync.dma_start(inpT[0][:, :], inpT_d[0])
            nc.sync.dma_start(inp8[0][:, :], inp8_d[0])
            nc.sync.dma_start(w1s[:, :], w1_d[:, :])
            nc.sync.dma_start(w8s[:, :], w8_d[:, :])
            nc.sync.dma_start(inpT[1][:, :], inpT_d[1])
            nc.sync.dma_start(inp8[1][:, :], inp8_d[1])
            nc.sync.dma_start(w2z[:, :], w2z_d[:, :])
            # slices 2..7 are DMA'd lazily inside the pair loop (two pairs
            # ahead of first use) so the first matmuls don't sit behind the
            # whole input stream on the DMA queue.

            with (
                tc.tile_pool(name="hs", bufs=HS_B) as hs,
                tc.tile_pool(name="h2p", bufs=H2P_B) as h2p,
                tc.tile_pool(name="psq", bufs=PSQ_B) as psq,
                tc.tile_pool(name="fin", bufs=FIN_B) as fin,
                tc.tile_pool(name="fint", bufs=FINT_B) as fint,
                tc.tile_pool(name="ps_g1", bufs=G1B, space="PSUM") as ps_g1,
                tc.tile_pool(name="ps_g2", bufs=G2B, space="PSUM") as ps_g2,
                tc.tile_pool(name="ps_p", bufs=PPB, space="PSUM") as ps_p,
            ):
                pending_fin = []
                pending_fin2 = []
                pending_fin3 = []
                rep_tiles = {}
                stage_tiles = {}

                def block_A(j, i):
                    """mm0 into g1 chunks (512-wide matmuls)."""
                    jH = j * H
                    g1_tiles = []
                    for c in range(NG1):
                        g1 = ps_g1.tile([128, G1N], F32, tag="g1")
                        for s in range(G1N // QT):
                            col = c * G1N + s * QT
                            nc.tensor.matmul(
                                g1[:, s * QT:(s + 1) * QT],
                                w0s[:, jH:jH + H],
                                inpT[i][:, col:col + QT],
                            )
                        g1_tiles.append(g1)
                    return g1_tiles

                def block_B(j, i, g2_tiles):
                    """leaky2 (ACT) + mm2 x4 -> pp for pair (j, i)."""
                    if PP_SHARE_G1:
                        pp = ps_g1.tile([128, G1N], F32, tag="g1",
                                        name="pp")[:, :QT]
                    else:
                        pp = ps_p.tile([128, QT], F32, tag="pp")
                    for c in range(NG2):
                        h2 = h2p.tile([128, G2N], BF16, tag="h2")
                        nc.scalar.activation(
                            h2[:, :], g2_tiles[c][:, :],
                            mybir.ActivationFunctionType.Prelu,
                            bias=0.0, alpha=SLOPE,
                        )
                        for s in range(G2N // QT):
                            t = (c * G2N) // QT + s
                            zc = (j * 4 + t) * H
                            nc.tensor.matmul(
                                pp[:, :],
                                w2z[:, zc:zc + H],
                                h2[:, s * QT:(s + 1) * QT],
                                start=(t == 0), stop=(t == 3),
                            )
                    return pp

                def emit_w01(j, i, g2, c):
                    w8v = w8s[:, :].rearrange(
                        "d (j s h) -> d j s h", j=E, s=2)
                    i8v = inp8[i][:, :].rearrange("d (s t) -> d s t", s=2)
                    for s in range(G2N // QT):
                        col = c * G2N + s * QT
                        nc.tensor.matmul(
                            g2[:, s * QT:(s + 1) * QT],
                            w8v[:, j, :, :],
                            i8v[:, :, col:col + QT],
                            start=True, stop=False,
                            perf_mode=mybir.MatmulPerfMode.DoubleRow,
                        )

                def block_C(j, i):
                    """g2 tile allocs (+ W01 DR matmuls unless W01_IN_D)."""
                    g2_tiles = []
                    for c in range(NG2):
                        g2 = ps_g2.tile([128, G2N], F32, tag="g2")
                        if not W01_IN_D:
                            emit_w01(j, i, g2, c)
                        g2_tiles.append(g2)
                    return g2_tiles

                def block_relu(g1_tiles):
                    r1_tiles = []
                    for c in range(NG1):
                        r1 = hs.tile([128, G1N], BF16, tag="r1")
                        n = ctr["relu"]
                        ctr["relu"] += 1
                        if RELU_ACT_MOD and n % RELU_ACT_MOD == 0:
                            nc.scalar.activation(
                                r1[:, :], g1_tiles[c][:, :],
                                mybir.ActivationFunctionType.Relu,
                            )
                        else:
                            nc.vector.tensor_scalar_max(
                                r1[:, :], g1_tiles[c][:, :], 0.0)
                        r1_tiles.append(r1)
                    return r1_tiles

                def block_D(j, i, g2_tiles, r1_tiles):
                    """mm1 x4 completing the g2 chunks."""
                    jH = j * H
                    for c in range(NG2):
                        if W01_IN_D:
                            emit_w01(j, i, g2_tiles[c], c)
                        for s in range(G2N // QT):
                            col = c * G2N + s * QT
                            rv = r1_tiles[col // G1N][
                                :, col % G1N:col % G1N + QT]
                            nc.tensor.matmul(
                                g2_tiles[c][:, s * QT:(s + 1) * QT],
                                w1s[:, jH:jH + H],
                                rv,
                                start=False, stop=True,
                            )

                def block_drain(j, i, pp):
                    """sum (DVE from PSUM), sq (ACT), sumsq (Pool)."""
                    acc_sl = (slice(None), slice(j * QT, (j + 1) * QT))
                    sq_s = 1.0 / math.sqrt(E - 1.0)
                    if i == 0:
                        nc.vector.tensor_copy(sum_acc[acc_sl], pp[:, :])
                        nc.scalar.activation(
                            sumsq_acc[acc_sl], pp[:, :],
                            mybir.ActivationFunctionType.Square,
                            scale=sq_s,
                        )
                    else:
                        nc.vector.scalar_tensor_tensor(
                            sum_acc[acc_sl], pp[:, :], 1.0,
                            sum_acc[acc_sl],
                            mybir.AluOpType.mult, mybir.AluOpType.add,
                        )
                        sq = psq.tile([128, QT], F32, tag="sq")
                        nc.scalar.activation(
                            sq[:, :], pp[:, :],
                            mybir.ActivationFunctionType.Square,
                            scale=sq_s,
                        )
                        nc.gpsimd.tensor_tensor(
                            sumsq_acc[acc_sl], sumsq_acc[acc_sl],
                            sq[:, :], mybir.AluOpType.add,
                        )

                def block_fin_slots(j, i):
                    if i == 1 and pending_fin:
                        pending_fin.pop(0)()
                    if i == 3 and pending_fin2:
                        pending_fin2.pop(0)()
                    if i == 5 and pending_fin3:
                        pending_fin3.pop(0)()
                    if i == 4:
                        rep = fin.tile([128, QT], F32, tag="rep")
                        for t in range(4):
                            nc.sync.dma_start(
                                rep[32 * t:32 * (t + 1), :],
                                statesT_d[j][:, t * QT:(t + 1) * QT],
                            )
                        rep_tiles[j] = rep
                    if i == E - 1:
                        finalize(j)

                def finalize(j):
                    acc_sl = (slice(None), slice(j * QT, (j + 1) * QT))

                    def fin_stage1(j=j, acc_sl=acc_sl):
                        # msq = (s*sum)^2 with s = 1/sqrt(E(E-1)):
                        # exactly the E/(E-1)*mean_delta^2 term of var
                        msq = fint.tile([128, QT], F32, tag="msq",
                                        name=f"msq{j}")
                        nc.scalar.activation(
                            msq[:, :], sum_acc[acc_sl],
                            mybir.ActivationFunctionType.Square,
                            scale=1.0 / math.sqrt(E * (E - 1.0)),
                        )
                        stage_tiles[j] = msq

                    def fin_stage2(j=j, acc_sl=acc_sl):
                        msq = stage_tiles[j]
                        rep = rep_tiles.pop(j)
                        mean_st = fint.tile([128, QT], F32, tag="mean_st",
                                            name=f"mean_st{j}")
                        if MEAN_ON_POOL:
                            mtmp = fint.tile([128, QT], F32, tag="mtmp",
                                             name=f"mtmp{j}")
                            nc.gpsimd.tensor_tensor(
                                mtmp[:, :], sum_acc[acc_sl], eighth[:, :],
                                mybir.AluOpType.mult,
                            )
                            nc.gpsimd.tensor_tensor(
                                mean_st[:, :], mtmp[:, :], rep[:, :],
                                mybir.AluOpType.add,
                            )
                        else:
                            nc.vector.scalar_tensor_tensor(
                                mean_st[:, :], sum_acc[acc_sl], 1.0 / E,
                                rep[:, :],
                                mybir.AluOpType.mult, mybir.AluOpType.add,
                            )
                        var_st = fint.tile([128, QT], F32, tag="var_st",
                                           name=f"var_st{j}")
                        nc.gpsimd.tensor_tensor(
                            var_st[:, :], sumsq_acc[acc_sl], msq[:, :],
                            mybir.AluOpType.subtract,
                        )
                        xm = fint.tile([128, QT], F32, tag="xm", name=f"xm{j}")
                        nc.vector.transpose(xm[:, :], mean_st[:, :])
                        stage_tiles[j] = (xm, var_st)

                    def fin_stage3(j=j):
                        xm, var_st = stage_tiles.pop(j)
                        xv = fint.tile([128, QT], F32, tag="xv", name=f"xv{j}")
                        nc.vector.transpose(xv[:, :], var_st[:, :])
                        for t in range(4):
                            tok = slice(t * QT, (t + 1) * QT)
                            prt = slice(32 * t, 32 * (t + 1))
                            nc.sync.dma_start(
                                mean_d[j][tok].rearrange(
                                    "(u p) k -> p u k", p=32),
                                xm[prt, :].rearrange(
                                    "p (u k) -> p u k", k=DS),
                            )
                            nc.sync.dma_start(
                                var_d[j][tok].rearrange(
                                    "(u p) k -> p u k", p=32),
                                xv[prt, :].rearrange(
                                    "p (u k) -> p u k", k=DS),
                            )

                    pending_fin.append(fin_stage1)
                    pending_fin2.append(fin_stage2)
                    pending_fin3.append(fin_stage3)

                # ---- software-pipelined emission (steady state):
                #   A(p) mm0 | [C-mm(p) if SWAP_BC] | B(p-1) leaky2+mm2
                #   [+drains(p-1) if DRAIN_IN_B] | C(p) W01+relu | D(p) mm1
                #   | [drains(p-1) if not DRAIN_IN_B] | fin slots(p-1)
                pairs = [(r, j, i) for r in range(repeat)
                         for j in range(E) for i in range(E)]
                prev = None
                for (r, j, i) in pairs:
                    if r == 0 and j == 0 and i + 2 < E:
                        nc.sync.dma_start(inpT[i + 2][:, :], inpT_d[i + 2])
                        nc.sync.dma_start(inp8[i + 2][:, :], inp8_d[i + 2])
                    g1t = block_A(j, i)
                    if SWAP_BC:
                        g2t = block_C(j, i)
                    if prev is not None:
                        pj, pi, pg2 = prev
                        pp_prev = block_B(pj, pi, pg2)
                        if DRAIN_IN_B:
                            block_drain(pj, pi, pp_prev)
                    if not SWAP_BC:
                        g2t = block_C(j, i)
                    r1t = block_relu(g1t)
                    block_D(j, i, g2t, r1t)
                    if prev is not None:
                        if not DRAIN_IN_B:
                            block_drain(pj, pi, pp_prev)
                        block_fin_slots(pj, pi)
                    prev = (j, i, g2t)
                pj, pi, pg2 = prev
                pp_prev = block_B(pj, pi, pg2)
                block_drain(pj, pi, pp_prev)
                block_fin_slots(pj, pi)

                for f in pending_fin:
                    f()
                pending_fin.clear()
                for f in pending_fin2:
                    f()
                pending_fin2.clear()
                for f in pending_fin3:
                    f()
                pending_fin3.clear()

    nc.compile()
    return nc


def build_kernel(zero_bias: bool, repeat: int = 1):
    if zero_bias:
        return build_kernel_v3(repeat=repeat)
    return build_kernel_v1(zero_bias, repeat=repeat)


# ---------------------------------------------------------------------------
# v1: generic fallback (non-zero biases), f32r matmuls. Unchanged from the
# previous session's baseline.
# ---------------------------------------------------------------------------

DVE_LK_MOD = 7
DVE_LK_SET = (1, 4)
PE_SUM = True


def build_kernel_v1(zero_bias: bool, repeat: int = 1):
    nc = bacc.Bacc("TRN2")

    states_d = nc.dram_tensor("states", [E, B_LOC, DS], F32, kind="ExternalInput")
    actions_d = nc.dram_tensor("actions", [E, B_LOC, DA], F32, kind="ExternalInput")
    w0_d = nc.dram_tensor("w0", [E, DIN, H], F32, kind="ExternalInput")
    b0_d = nc.dram_tensor("b0", [E, H], F32, kind="ExternalInput")
    w1_d = nc.dram_tensor("w1", [E, H, H], F32, kind="ExternalInput")
    b1_d = nc.dram_tensor("b1", [E, H], F32, kind="ExternalInput")
    w2_d = nc.dram_tensor("w2", [E, H, DS], F32, kind="ExternalInput")
    b2_d = nc.dram_tensor("b2", [E, DS], F32, kind="ExternalInput")
    mean_d = nc.dram_tensor("mean_out", [E, B_LOC, DS], F32, kind="ExternalOutput")
    var_d = nc.dram_tensor("var_out", [E, B_LOC, DS], F32, kind="ExternalOutput")

    ident_d = nc.inline_tensor(np.eye(128, dtype=np.float32), name="ident")

    pe_sum = PE_SUM and zero_bias
    lk_ctr = [0]

    def leaky512(out_ap, psum_ap, pool, bias_ap):
        n = lk_ctr[0]
        lk_ctr[0] += 1
        if bias_ap is None and (n % DVE_LK_MOD) in DVE_LK_SET:
            t = pool.tile([128, QT], F32, tag="lk")
            nc.vector.tensor_scalar_mul(t[:, :], psum_ap, SLOPE)
            nc.vector.tensor_tensor(out_ap, t[:, :], psum_ap, mybir.AluOpType.max)
        else:
            nc.scalar.activation(
                out_ap, psum_ap, mybir.ActivationFunctionType.Prelu,
                bias=0.0 if bias_ap is None else bias_ap, alpha=SLOPE,
            )

    with tile.TileContext(nc) as tc:
        with (
            tc.tile_pool(name="wpool", bufs=1) as wpool,
            tc.tile_pool(name="big", bufs=1) as big,
            tc.tile_pool(name="io", bufs=1) as io,
            tc.tile_pool(name="hs", bufs=5) as hs,
            tc.tile_pool(name="fin", bufs=2) as fin,
        ):
            w0s = wpool.tile([DIN, E * H], F32R)
            w1s = wpool.tile([H, E * H], F32R)
            w2z = wpool.tile([H, E * 4 * H], F32R)
            ident = wpool.tile([128, 128], F32)

            nc.gpsimd.memset(w2z[:, :].bitcast(F32), 0.0)
            nc.sync.dma_start(ident[:, :], ident_d[:, :])
            nat_tiles = []
            for i in range(E):
                nat = io.tile([128, 16 * DIN], F32, tag=f"nat{i}",
                              name=f"nat{i}")
                natv = nat[:, :].rearrange("p (m d) -> p m d", m=16)
                nc.sync.dma_start(
                    natv[:, :, 0:DS],
                    states_d[i].rearrange("(m p) d -> p m d", p=128),
                )
                nc.sync.dma_start(
                    natv[:, :, DS:DIN],
                    actions_d[i].rearrange("(m p) d -> p m d", p=128),
                )
                nat_tiles.append(nat)
            nc.sync.dma_start(
                w0s[:, :].rearrange("d (j h) -> d j h", j=E),
                w0_d[:, :, :].rearrange("j d h -> d j h").bitcast(F32R),
            )
            nc.sync.dma_start(
                w1s[:, :].rearrange("d (j h) -> d j h", j=E),
                w1_d[:, :, :].rearrange("j d h -> d j h").bitcast(F32R),
            )
            w2zv = w2z[:, :].rearrange("d (j q) -> d j q", j=E)
            for t in range(4):
                nc.sync.dma_start(
                    w2zv[:, :, 160 * t: 160 * t + DS],
                    w2_d[:, :, :].rearrange("j d k -> d j k").bitcast(F32R),
                )
            if not zero_bias:
                b0s = wpool.tile([H, E], F32)
                b1s = wpool.tile([H, E], F32)
                b2r = wpool.tile([H, E], F32)
                nc.sync.dma_start(b0s[:, :], b0_d[:, :].rearrange("j h -> h j"))
                nc.sync.dma_start(b1s[:, :], b1_d[:, :].rearrange("j h -> h j"))
                for t in range(4):
                    nc.sync.dma_start(
                        b2r[32 * t: 32 * (t + 1), :],
                        b2_d[:, :].rearrange("j k -> k j"),
                    )

            sum_acc = None if pe_sum else big.tile([128, E * QT], F32)
            sumsq_acc = big.tile([128, E * QT], F32)
            inpT = [
                big.tile([DIN, B_LOC], F32R, tag=f"inpT{i}", name=f"inpT{i}")
                for i in range(E)
            ]

            def make_input_emitter(tp_psum):
                def emit_input_phase(i):
                    nat = nat_tiles[i]
                    for g in range(NQ):
                        pt = tp_psum.tile([DIN, QT], F32, tag="pt", name=f"pt{i}_{g}")
                        for m in range(4):
                            mm = g * 4 + m
                            nc.tensor.transpose(
                                pt[:, m * 128:(m + 1) * 128],
                                nat[:, mm * DIN:(mm + 1) * DIN],
                                ident[:, :],
                            )
                        if g % 2 == 0:
                            nc.vector.tensor_copy(
                                inpT[i][:, g * QT:(g + 1) * QT], pt[:, :]
                            )
                        else:
                            nc.scalar.copy(
                                inpT[i][:, g * QT:(g + 1) * QT], pt[:, :]
                            )
                return emit_input_phase

            with tc.tile_pool(name="tp_psum", bufs=3, space="PSUM") as tp_psum:
                emit = make_input_emitter(tp_psum)
                for i in range(E):
                    emit(i)

            with (
                tc.tile_pool(name="ps_h1", bufs=2, space="PSUM") as ps_h1,
                tc.tile_pool(name="ps_h2", bufs=2, space="PSUM") as ps_h2,
                tc.tile_pool(name="ps_p", bufs=1, space="PSUM") as ps_p,
                tc.tile_pool(name="ps_s", bufs=1, space="PSUM") as ps_s,
            ):
              pending_fin = []
              for _rep in range(repeat):
                for j in range(E):
                    jH = j * H
                    psum_sum = None
                    if pe_sum:
                        psum_sum = ps_s.tile([128, QT], F32, tag="psum",
                                             name="psum_sum")
                    for i in range(E):
                        pp = ps_p.tile([128, QT], F32, tag="pp")
                        for c in range(NCH):
                            h2p = ps_h2.tile([128, CH], F32, tag="h2p")
                            for s in range(CH // QT):
                                base = c * CH + s * QT
                                h1p = ps_h1.tile([128, QT], F32, tag="h1p")
                                nc.tensor.matmul(
                                    h1p[:, :],
                                    w0s[:, jH:jH + H],
                                    inpT[i][:, base:base + QT],
                                )
                                h1s = hs.tile([128, QT], F32R, tag="h1s")
                                leaky512(
                                    h1s[:, :], h1p[:, :], hs,
                                    None if zero_bias else b0s[:, j:j + 1],
                                )
                                nc.tensor.matmul(
                                    h2p[:, s * QT:(s + 1) * QT],
                                    w1s[:, jH:jH + H],
                                    h1s[:, :],
                                )
                            h2s = hs.tile([128, CH], F32R, tag="h2s")
                            for s in range(CH // QT):
                                sl = (slice(None), slice(s * QT, (s + 1) * QT))
                                leaky512(
                                    h2s[sl], h2p[sl], hs,
                                    None if zero_bias else b1s[:, j:j + 1],
                                )
                            for t2 in range(CH // QT):
                                T = c * (CH // QT) + t2
                                zc = (j * 4 + T) * H
                                rhs = h2s[:, t2 * QT:(t2 + 1) * QT]
                                nc.tensor.matmul(
                                    pp[:, :], w2z[:, zc:zc + H], rhs,
                                    start=(T == 0), stop=(T == 3),
                                )
                                if pe_sum:
                                    nc.tensor.matmul(
                                        psum_sum[:, :], w2z[:, zc:zc + H], rhs,
                                        start=(i == 0 and T == 0),
                                        stop=(i == E - 1 and T == 3),
                                        skip_group_check=True,
                                    )
                        acc_sl = (slice(None), slice(j * QT, (j + 1) * QT))
                        sq_bias = 0.0 if zero_bias else b2r[:, j:j + 1]
                        if not pe_sum:
                            if i == 0:
                                nc.vector.tensor_copy(sum_acc[acc_sl], pp[:, :])
                            else:
                                nc.vector.tensor_tensor(
                                    sum_acc[acc_sl], sum_acc[acc_sl], pp[:, :],
                                    mybir.AluOpType.add,
                                )
                        if i == 1 and pending_fin:
                            pending_fin.pop(0)()
                        if i == 4:
                            rep = fin.tile([128, QT], F32, tag="rep")
                            for t in range(4):
                                nc.sync.dma_start(
                                    rep[32 * t:32 * (t + 1), :],
                                    inpT[j][:DS, t * QT:(t + 1) * QT
                                            ].bitcast(F32),
                                )
                        if i == 0:
                            nc.scalar.activation(
                                sumsq_acc[acc_sl], pp[:, :],
                                mybir.ActivationFunctionType.Square,
                                bias=sq_bias,
                            )
                        else:
                            sq = hs.tile([128, QT], F32, tag="sq")
                            nc.scalar.activation(
                                sq[:, :], pp[:, :],
                                mybir.ActivationFunctionType.Square,
                                bias=sq_bias,
                            )
                            nc.gpsimd.tensor_tensor(
                                sumsq_acc[acc_sl], sumsq_acc[acc_sl], sq[:, :],
                                mybir.AluOpType.add,
                            )

                    acc_sl = (slice(None), slice(j * QT, (j + 1) * QT))
                    m_t = fin.tile([128, QT], F32, tag="m_t")
                    msrc = psum_sum[:, :] if pe_sum else sum_acc[acc_sl]
                    if zero_bias:
                        nc.vector.tensor_scalar(
                            m_t[:, :], msrc, 1.0 / E, None, mybir.AluOpType.mult,
                        )
                    else:
                        nc.vector.tensor_scalar(
                            m_t[:, :], msrc, 1.0 / E, b2r[:, j:j + 1],
                            mybir.AluOpType.mult, mybir.AluOpType.add,
                        )

                    def fin_tail(j=j, m_t=m_t, rep=rep, acc_sl=acc_sl):
                        mean_st = fin.tile([128, QT], F32, tag="mean_st",
                                           name=f"mean_st{j}")
                        nc.gpsimd.tensor_tensor(
                            mean_st[:, :], m_t[:, :], rep[:, :],
                            mybir.AluOpType.add
                        )
                        msq = fin.tile([128, QT], F32, tag="msq",
                                       name=f"msq{j}")
                        nc.scalar.activation(
                            msq[:, :], m_t[:, :],
                            mybir.ActivationFunctionType.Square,
                            scale=math.sqrt(E / (E - 1.0)),
                        )
                        var_st = fin.tile([128, QT], F32, tag="var_st",
                                          name=f"var_st{j}")
                        nc.vector.scalar_tensor_tensor(
                            var_st[:, :], sumsq_acc[acc_sl], 1.0 / (E - 1.0),
                            msq[:, :],
                            mybir.AluOpType.mult, mybir.AluOpType.subtract,
                        )
                        xm = fin.tile([128, QT], F32, tag="xm", name=f"xm{j}")
                        nc.vector.transpose(xm[:, :], mean_st[:, :])
                        xv = fin.tile([128, QT], F32, tag="xv", name=f"xv{j}")
                        nc.vector.transpose(xv[:, :], var_st[:, :])
                        for t in range(4):
                            tok = slice(t * QT, (t + 1) * QT)
                            prt = slice(32 * t, 32 * (t + 1))
                            nc.sync.dma_start(
                                mean_d[j][tok].rearrange("(u p) k -> p u k", p=32),
                                xm[prt, :].rearrange("p (u k) -> p u k", k=DS),
                            )
                            nc.sync.dma_start(
                                var_d[j][tok].rearrange("(u p) k -> p u k", p=32),
                                xv[prt, :].rearrange("p (u k) -> p u k", k=DS),
                            )

                    pending_fin.append(fin_tail)

            for f in pending_fin:
                f()
            pending_fin.clear()

    nc.compile()
    return nc


_NC_CACHE = {}


def make_in_maps(inputs, zero_bias=True):
    """Per-core input dicts (host-prepacked bf16/fp8 operands for v3)."""
    W0 = np.ascontiguousarray(inputs["W0"], dtype=np.float32)
    W1 = np.ascontiguousarray(inputs["W1"], dtype=np.float32)
    W2 = np.ascontiguousarray(inputs["W2"], dtype=np.float32)
    states = np.ascontiguousarray(inputs["states"], dtype=np.float32)
    actions = np.ascontiguousarray(inputs["actions"], dtype=np.float32)
    in_maps = []
    if zero_bias:
        import ml_dtypes
        BF = ml_dtypes.bfloat16
        # stationaries, transposed + flattened on host
        w0s = np.ascontiguousarray(
            W0.transpose(1, 0, 2).reshape(DIN, E * H).astype(BF))
        w1s = np.ascontiguousarray(
            (0.99 * W1).transpose(1, 0, 2).reshape(H, E * H).astype(BF))
        # zero-padded W2 blocks: block (j, t) at cols j*512 + 160*t
        w2z = np.zeros((H, E, 4 * H), BF)
        for t in range(4):
            w2z[:, :, 160 * t:160 * t + DS] = (
                W2.transpose(1, 0, 2).astype(BF))
        w2z = np.ascontiguousarray(w2z.reshape(H, E * 4 * H))
        # fused fp8 weight for the 0.01 linear term of leaky on layer 1
        w01 = np.einsum(
            "jdh,jhk->jdk", W0.astype(np.float64), W1.astype(np.float64)
        ).astype(np.float32)
        w8 = np.zeros((DIN, E, 2, H), ml_dtypes.float8_e4m3)
        w8[:, :, 0, :] = w01.transpose(1, 0, 2).astype(ml_dtypes.float8_e4m3)
        w8 = np.ascontiguousarray(w8.reshape(DIN, E * 2 * H))
        inp = np.concatenate([states, actions], axis=2)  # (E, B, DIN)
        inpTg = inp.transpose(0, 2, 1)                   # (E, DIN, B)
        statesTg = states.transpose(0, 2, 1)             # (E, DS, B)
        for c in range(N_CORES):
            sl = slice(c * B_LOC, (c + 1) * B_LOC)
            inp8 = np.zeros((E, DIN, 2, B_LOC), ml_dtypes.float8_e5m2)
            inp8[:, :, 0, :] = (0.01 * inpTg[:, :, sl]
                                ).astype(ml_dtypes.float8_e5m2)
            in_maps.append({
                "inpT": np.ascontiguousarray(inpTg[:, :, sl].astype(BF)),
                "statesT": np.ascontiguousarray(statesTg[:, :, sl]),
                "w0s": w0s, "w1s": w1s, "w2z": w2z, "w8": w8,
                "inp8": np.ascontiguousarray(
                    inp8.reshape(E, DIN, 2 * B_LOC)),
            })
    else:
        b0 = np.ascontiguousarray(inputs["b0"], dtype=np.float32)
        b1 = np.ascontiguousarray(inputs["b1"], dtype=np.float32)
        b2 = np.ascontiguousarray(inputs["b2"], dtype=np.float32)
        for c in range(N_CORES):
            sl = slice(c * B_LOC, (c + 1) * B_LOC)
            in_maps.append({
                "states": np.ascontiguousarray(states[:, sl, :]),
                "actions": np.ascontiguousarray(actions[:, sl, :]),
                "w0": W0, "b0": b0, "w1": W1, "b1": b1, "w2": W2, "b2": b2,
            })
    return in_maps


def kernel(states, actions, W0, b0, W1, b1, W2, b2):
    states = np.ascontiguousarray(states, dtype=np.float32)
    actions = np.ascontiguousarray(actions, dtype=np.float32)
    W0 = np.ascontiguousarray(W0, dtype=np.float32)
    W1 = np.ascontiguousarray(W1, dtype=np.float32)
    W2 = np.ascontiguousarray(W2, dtype=np.float32)
    b0 = np.ascontiguousarray(b0, dtype=np.float32)
    b1 = np.ascontiguousarray(b1, dtype=np.float32)
    b2 = np.ascontiguousarray(b2, dtype=np.float32)

    zb = not (b0.any() or b1.any() or b2.any())
    if zb not in _NC_CACHE:
        _NC_CACHE[zb] = build_kernel(zb)
    nc = _NC_CACHE[zb]

    in_maps = make_in_maps(
        {"states": states, "actions": actions, "W0": W0, "b0": b0,
         "W1": W1, "b1": b1, "W2": W2, "b2": b2}, zb)

    res = run_bass_kernel_spmd(nc, in_maps, list(range(N_CORES)))
    mean = np.concatenate([r["mean_out"] for r in res.results], axis=1)
    var = np.concatenate([r["var_out"] for r in res.results], axis=1)
    return mean, var
